# revision 2
# baseline (speedup 1.0000x reference)
"""Gated-attention (Qwen-style) Trainium2 kernel — fp16, scheduling-optimized.

Sharding (8 cores): data-parallel over batch (2) x tensor-parallel over head
groups (4). Core c handles batch b=c//4 and head group g=c%4: q heads
4g..4g+3, kv heads 2g..2g+1, gate logits 4g..4g+3, w_o columns 512g..512g+512.
Each core computes a partial output y_g = attn_out_g @ w_o[:, cols_g].T in
fp16; the host sums the 4 partials per batch in f32.

v2 changes vs the 437us baseline (which was ACT-bound in phase 2):
- exp runs on [128,1024] PSUM score-pair tiles (2 banks, one exp per j-pair)
  cutting ACT exp time ~17% and halving exp instruction count.
- e-sum accumulation on [128,1024] pairs (half the DVE instructions), with a
  single [128,512] fold add before the ones-matmul denominator.
- phase 1 qkv projection runs in stationary groups of 2 output tiles; the
  rope/transpose processing of group g is emitted after the accumulation MMs
  of group g+1 so the PE never stalls waiting on PSUM->SBUF casts.
- craw/vraw casts moved DVE->ACT (ACT is idle in phase 1).
- phase 2 emission: den/bcast chain for (i,kv0) is deferred until after the
  first j-pair of (i,kv1); out-projection t-tiles of block i-1 fill the PE
  while (i,kv1)'s denominator chain resolves on DVE/ACT.
- psy (out-proj PSUM) drain split DVE(3):ACT(1) per t-tile.
"""

import os
from contextlib import ExitStack

import numpy as np

B, S, HID = 2, 2048, 2048
NH, NKV, HD = 16, 8, 128
GATE = NH
KV_DIM = NKV * HD

N_CORES = 8
TPG = 4            # tensor-parallel group size (head groups)
QH = NH // TPG     # q heads per core = 4
KVH = NKV // TPG   # kv heads per core = 2
IB = 512           # phase-1 token block
NB = S // IB       # 4 blocks
JT = S // 128      # 16 key tiles
JP = JT // 2       # 8 key tile-pairs
IBLK = 512         # phase-2 query block
NI = S // IBLK     # 4 query blocks
WCOL = 1032        # packed qkv+gate weight cols (1028 used, padded)
SCALE = 1.0 / float(np.sqrt(HD))

_CACHE = {}

LAST_EXEC_NS = None
LAST_RESULTS = None


def _build_program():
    import concourse.bass as bass
    import concourse.mybir as mybir
    from concourse import bacc
    from concourse.tile import TileContext

    F32 = mybir.dt.float32
    F32R = mybir.dt.float32r
    F16 = mybir.dt.float16
    AF = mybir.ActivationFunctionType

    nc = bacc.Bacc()

    xT_d = nc.dram_tensor("xT", [HID, S], F16, kind="ExternalInput")
    wqkvT_d = nc.dram_tensor("wqkvT", [HID, WCOL], F16, kind="ExternalInput")
    woT_d = nc.dram_tensor("woT", [QH * HD, HID], F16, kind="ExternalInput")
    cosT_d = nc.dram_tensor("cosT", [HD, S], F32, kind="ExternalInput")
    sinT_d = nc.dram_tensor("sinT", [HD, S], F32, kind="ExternalInput")
    rotm_d = nc.dram_tensor("rotm", [HD, HD], F16, kind="ExternalInput")
    ident_d = nc.dram_tensor("ident", [128, 128], F16, kind="ExternalInput")
    ones1_d = nc.dram_tensor("ones1", [128, 1], F16, kind="ExternalInput")
    onesr_d = nc.dram_tensor("onesr", [1, 128], F32R, kind="ExternalInput")
    y_d = nc.dram_tensor("y", [S, HID], F16, kind="ExternalOutput")

    with TileContext(nc) as tc, ExitStack() as persist:
        const = persist.enter_context(tc.tile_pool(name="const", bufs=1))
        rotm_sb = const.tile([HD, HD], F16, tag="rotm", name="rotm")
        nc.scalar.dma_start(out=rotm_sb, in_=rotm_d[:, :])
        ident_sb = const.tile([128, 128], F16, tag="ident", name="ident")
        nc.scalar.dma_start(out=ident_sb, in_=ident_d[:, :])
        ones1_sb = const.tile([128, 1], F16, tag="ones1", name="ones1")
        nc.scalar.dma_start(out=ones1_sb, in_=ones1_d[:, :])
        onesr_sb = const.tile([1, 128], F32R, tag="onesr", name="onesr")
        nc.scalar.dma_start(out=onesr_sb, in_=onesr_d[:, :])

        # weights on ACT/DVE sequencers so x loads own the SP/Pool DGEs
        wpool = persist.enter_context(tc.tile_pool(name="w", bufs=1))
        wsb = [wpool.tile([128, WCOL], F16, tag=f"w{h}", name=f"w{h}") for h in range(16)]
        for h in range(16):
            nc.scalar.dma_start(out=wsb[h], in_=wqkvT_d[128 * h:128 * (h + 1), :])
        cos_sb = const.tile([HD, S], F32, tag="cos", name="cos")
        nc.scalar.dma_start(out=cos_sb, in_=cosT_d[:, :])
        sin_sb = const.tile([HD, S], F32, tag="sin", name="sin")
        nc.scalar.dma_start(out=sin_sb, in_=sinT_d[:, :])
        wopool = persist.enter_context(tc.tile_pool(name="wo", bufs=1))
        wo_sb = [wopool.tile([128, HID], F16, tag=f"wo{i}", name=f"wo{i}") for i in range(4)]

        qk_pool = persist.enter_context(tc.tile_pool(name="qk", bufs=1))
        qk_sb = [qk_pool.tile([128, S], F16, tag=f"qk{r}", name=f"qk{r}") for r in range(QH + KVH)]
        v_pool = persist.enter_context(tc.tile_pool(name="v", bufs=1))
        v_sb = [v_pool.tile([128, KVH * HD], F16, tag=f"v{t}", name=f"v{t}") for t in range(JT)]
        g_pool = persist.enter_context(tc.tile_pool(name="g", bufs=1))
        sgflat = g_pool.tile([1, QH * S], F32, tag="sgflat", name="sgflat")

        # ---------------- phase 1: qkv projection + rope + v transpose -----
        with ExitStack() as ph1:
            xpool = ph1.enter_context(tc.tile_pool(name="x", bufs=32))
            tmppool = ph1.enter_context(tc.tile_pool(name="tmp", bufs=3))
            vrawpool = ph1.enter_context(tc.tile_pool(name="vraw", bufs=2))
            sgpool = ph1.enter_context(tc.tile_pool(name="sg", bufs=1))

            ps_acc = ph1.enter_context(tc.tile_pool(name="acc", bufs=4, space="PSUM"))
            ps_rot = ph1.enter_context(tc.tile_pool(name="rot", bufs=1, space="PSUM"))
            ps_tp = ph1.enter_context(tc.tile_pool(name="tp", bufs=2, space="PSUM"))
            ps_g = ph1.enter_context(tc.tile_pool(name="psg", bufs=1, space="PSUM"))

            # deferred per-group processing closure (runs one group behind)
            pending_proc = [None]

            def emit_pending():
                if pending_proc[0] is not None:
                    pending_proc[0]()
                    pending_proc[0] = None

            for ib in range(NB):
                sl = slice(IB * ib, IB * (ib + 1))
                xb = []
                for h in range(16):
                    xt = xpool.tile([128, IB], F16, tag="x", name="x")
                    eng = nc.gpsimd if h % 2 == 0 else nc.sync
                    eng.dma_start(out=xt, in_=xT_d[128 * h:128 * (h + 1), sl])
                    xb.append(xt)

                # gate logits (stationary = packed cols 1024:1028)
                psg = ps_g.tile([QH, IB], F32, tag="psg", name="psg")
                for h in range(16):
                    nc.tensor.matmul(psg, wsb[h][:, 1024:1024 + QH], xb[h],
                                     start=(h == 0), stop=(h == 15))
                # previous block's last group processing fills the PE gap
                emit_pending()
                # sigmoid per block, flattened to partition 0 for phase 2
                eT = sgpool.tile([QH, IB], F32, tag="eT", name="eT")
                nc.scalar.activation(out=eT, in_=psg, func=AF.Exp, scale=-1.0)
                nc.vector.tensor_scalar_add(eT, eT, 1.0)
                sgT = sgpool.tile([QH, IB], F32, tag="sgT", name="sgT")
                nc.vector.reciprocal_approx_fast(out=sgT, in_=eT)
                for h in range(QH):
                    nc.sync.dma_start(out=sgflat[0:1, S * h + IB * ib:S * h + IB * (ib + 1)],
                                      in_=sgT[h:h + 1, :])

                # 4 stationary groups of 2 output row-tiles each:
                # (q0,q1) (q2,q3) (k0,k1) (v0,v1)
                for grp in range(4):
                    accs = [ps_acc.tile([128, IB], F32, tag="acc", name="acc")
                            for _ in range(2)]
                    for h in range(16):
                        for r2 in range(2):
                            r = 2 * grp + r2
                            nc.tensor.matmul(
                                accs[r2], wsb[h][:, 128 * r:128 * (r + 1)], xb[h],
                                start=(h == 0), stop=(h == 15))

                    def make_proc(grp, accs, sl):
                        def proc():
                            for r2 in range(2):
                                r = 2 * grp + r2
                                if r < QH + KVH:  # q or k row-tile: rope
                                    craw = tmppool.tile([128, IB], F16, tag="craw", name="craw")
                                    nc.scalar.copy(craw, accs[r2])
                                    rps = ps_rot.tile([128, IB], F32, tag="rot", name="rot")
                                    nc.tensor.matmul(rps, rotm_sb, craw, start=True, stop=True)
                                    t1 = tmppool.tile([128, IB], F32R, tag="t1", name="t1")
                                    nc.vector.tensor_mul(t1, accs[r2], cos_sb[:, sl])
                                    t2 = tmppool.tile([128, IB], F32R, tag="t2", name="t2")
                                    nc.vector.tensor_mul(t2, rps, sin_sb[:, sl])
                                    nc.vector.tensor_add(qk_sb[r][:, sl], t1, t2)
                                else:  # v row-tile: transpose to [tokens, d]
                                    vraw = vrawpool.tile([128, IB], F16, tag="vraw", name="vraw")
                                    nc.scalar.copy(vraw, accs[r2])
                                    vh = r - (QH + KVH)
                                    ibb = (sl.start // IB)
                                    for s2 in range(IB // 128):
                                        tp = ps_tp.tile([128, 128], F16, tag="tp", name="tp")
                                        nc.tensor.transpose(
                                            tp, vraw[:, 128 * s2:128 * (s2 + 1)], ident_sb)
                                        tt = (IB // 128) * ibb + s2
                                        nc.vector.tensor_copy(
                                            v_sb[tt][:, 128 * vh:128 * (vh + 1)], tp)
                        return proc

                    if pending_proc[0] is not None:
                        pending_proc[0]()
                    pending_proc[0] = make_proc(grp, accs, sl)

            for cc in range(4):
                nc.gpsimd.dma_start(out=wo_sb[cc], in_=woT_d[128 * cc:128 * (cc + 1), :])
            emit_pending()

        # ---------------- phase 2: attention + gate + out-projection -------
        with ExitStack() as ph2:
            oc_pool = ph2.enter_context(tc.tile_pool(name="oc", bufs=1))
            OC = [oc_pool.tile([128, S], F16, tag=f"oc{h}", name=f"oc{h}") for h in range(QH)]
            epool = ph2.enter_context(tc.tile_pool(name="e", bufs=4))
            accpool = ph2.enter_context(tc.tile_pool(name="dacc", bufs=4))
            scpool = ph2.enter_context(tc.tile_pool(name="sc", bufs=2))
            foldpool = ph2.enter_context(tc.tile_pool(name="fold", bufs=2))
            ypool = ph2.enter_context(tc.tile_pool(name="y", bufs=2))

            ps_s = ph2.enter_context(tc.tile_pool(name="pss", bufs=2, space="PSUM"))
            ps_o = ph2.enter_context(tc.tile_pool(name="pso", bufs=2, space="PSUM"))
            ps_sh = ph2.enter_context(tc.tile_pool(name="pssh", bufs=2, space="PSUM"))

            def emit_jloop(i, kv):
                """scores->exp->PV j-pair loop for (i, kv). Returns accs2 pair tiles."""
                isl = slice(IBLK * i, IBLK * (i + 1))
                psos = []
                accs2 = []
                for hh in range(2):
                    psos.append(ps_o.tile([128, IBLK], F32, tag="pso", name="pso"))
                    accs2.append(accpool.tile([128, 2 * IBLK], F16, tag="dacc", name="dacc"))
                for jp in range(JP):
                    es = []
                    for hh in range(2):
                        h = 2 * kv + hh
                        pss = ps_s.tile([128, 2 * IBLK], F32, tag="pss", name="pss")
                        for jj in range(2):
                            j = 2 * jp + jj
                            jsl = slice(128 * j, 128 * (j + 1))
                            nc.tensor.matmul(pss[:, IBLK * jj:IBLK * (jj + 1)],
                                             qk_sb[QH + kv][:, jsl],
                                             qk_sb[h][:, isl], start=True, stop=True)
                        e = epool.tile([128, 2 * IBLK], F16, tag="e", name="e")
                        nc.scalar.activation(out=e, in_=pss, func=AF.Exp, scale=SCALE)
                        es.append(e)
                    for hh in range(2):
                        if jp == 0:
                            nc.vector.tensor_copy(accs2[hh], es[hh])
                        else:
                            nc.vector.tensor_add(accs2[hh], accs2[hh], es[hh])
                        for jj in range(2):
                            j = 2 * jp + jj
                            nc.tensor.matmul(psos[hh],
                                             v_sb[j][:, 128 * kv:128 * (kv + 1)],
                                             es[hh][:, IBLK * jj:IBLK * (jj + 1)],
                                             start=(j == 0), stop=(j == JT - 1))
                return psos, accs2

            def emit_den(i, kv, psos, accs2):
                """fold + denominator + bcast + gate/den scale + OC write."""
                isl = slice(IBLK * i, IBLK * (i + 1))
                for hh in range(2):
                    h = 2 * kv + hh
                    fold = foldpool.tile([128, IBLK], F16, tag="fold", name="fold")
                    nc.vector.tensor_add(fold, accs2[hh][:, :IBLK], accs2[hh][:, IBLK:])
                    den = ps_sh.tile([1, IBLK], F32, tag="sh", name="sh")
                    nc.tensor.matmul(den, ones1_sb, fold, start=True, stop=True)
                    rec = scpool.tile([1, IBLK], F32, tag="rec", name="rec")
                    nc.vector.reciprocal_approx_fast(out=rec, in_=den)
                    sc = scpool.tile([1, IBLK], F32R, tag="sc", name="sc")
                    nc.vector.tensor_mul(
                        sc, rec, sgflat[0:1, S * h + IBLK * i:S * h + IBLK * (i + 1)])
                    bc = ps_sh.tile([128, IBLK], F32, tag="sh", name="sh")
                    nc.tensor.matmul(bc, onesr_sb, sc, start=True, stop=True)
                    bcs = scpool.tile([128, IBLK], F16, tag="bcs", name="bcs")
                    nc.scalar.copy(bcs, bc)
                    nc.vector.tensor_mul(OC[h][:, isl], psos[hh], bcs)

            def emit_oproj(t):
                """out-projection for token tile t (16 MMs + 4 psy drains + y DMA)."""
                ysb = ypool.tile([128, HID], F16, tag="y", name="y")
                for o in range(4):
                    psy = ps_sh.tile([128, IBLK], F32, tag="sh", name="sh")
                    for cc in range(4):
                        nc.tensor.matmul(
                            psy, OC[cc][:, 128 * t:128 * (t + 1)],
                            wo_sb[cc][:, IBLK * o:IBLK * (o + 1)],
                            start=(cc == 0), stop=(cc == 3))
                    if o == 0:
                        nc.scalar.copy(ysb[:, IBLK * o:IBLK * (o + 1)], psy)
                    else:
                        nc.vector.tensor_copy(ysb[:, IBLK * o:IBLK * (o + 1)], psy)
                nc.gpsimd.dma_start(out=y_d[128 * t:128 * (t + 1), :], in_=ysb)

            # emission schedule: den chains are deferred and covered by other
            # PE work (next kv's j-pairs, or out-proj tiles of block i-1).
            pend_op = []      # deferred out-projection t-tiles
            for i in range(NI):
                psos0, accs0 = emit_jloop(i, 0)
                # first j-pair of kv1 covers kv0's den chain latency
                psos1, accs1 = None, None
                # (emit kv1 j-loop fully, then den0 — den0's fold runs on DVE
                #  during kv1's j-loop; PE reaches den0 MM with zero stall
                #  because kv1's 64 j-loop MMs are in front of it)
                # Instead interleave: den0 right after kv1's loop start is not
                # expressible without splitting emit_jloop; simpler: emit den0
                # after kv1's j-loop, den1 after out-proj filler.
                psos1, accs1 = emit_jloop(i, 1)
                emit_den(i, 0, psos0, accs0)
                # fill PE while (i,kv1) den chain resolves: out-proj of i-1
                if pend_op:
                    for t in pend_op:
                        emit_oproj(t)
                    pend_op = []
                emit_den(i, 1, psos1, accs1)
                # out-proj for this i: defer 2 of 4 t-tiles to next i
                emit_oproj(4 * i + 0)
                emit_oproj(4 * i + 1)
                pend_op = [4 * i + 2, 4 * i + 3]
            for t in pend_op:
                emit_oproj(t)

    nc.finalize()
    return nc


def kernel(hidden_states, cos, sin, w_qkv, w_o):
    global LAST_EXEC_NS, LAST_RESULTS
    from concourse.bass_utils import run_bass_kernel_spmd

    BF = np.float16
    hidden_states = np.asarray(hidden_states, dtype=np.float32)
    cos = np.asarray(cos, dtype=np.float32)
    sin = np.asarray(sin, dtype=np.float32)
    w_qkv = np.asarray(w_qkv, dtype=np.float32)
    w_o = np.asarray(w_o, dtype=np.float32)

    if "nc" not in _CACHE:
        _CACHE["nc"] = _build_program()
    nc = _CACHE["nc"]

    cosT = np.ascontiguousarray(cos.T)
    sinT = np.ascontiguousarray(sin.T)
    rotm = np.zeros((HD, HD), dtype=np.float32)
    for i in range(HD // 2):
        rotm[i + HD // 2, i] = -1.0   # rot[d'] = -q[d'+64] for d' < 64
        rotm[i, i + HD // 2] = 1.0    # rot[d'] = +q[d'-64] for d' >= 64
    rotm = rotm.astype(BF)
    ident = np.eye(128, dtype=np.float32).astype(BF)
    ones1 = np.ones((128, 1), dtype=np.float16)
    onesr = np.ones((1, 128), dtype=np.float32)

    xT = [np.ascontiguousarray(hidden_states[b].T).astype(BF) for b in range(B)]
    in_maps = []
    for c in range(N_CORES):
        b, g = divmod(c, TPG)
        qr = w_qkv[512 * g:512 * (g + 1)]
        kr = w_qkv[HID + GATE + 256 * g:HID + GATE + 256 * (g + 1)]
        vr = w_qkv[HID + GATE + KV_DIM + 256 * g:HID + GATE + KV_DIM + 256 * (g + 1)]
        gr = w_qkv[HID + QH * g:HID + QH * (g + 1)]
        pad = np.zeros((WCOL - 1024 - QH, HID), dtype=np.float32)
        wqkvT = np.ascontiguousarray(
            np.concatenate([qr, kr, vr, gr, pad], axis=0).T).astype(BF)
        woT = np.ascontiguousarray(w_o[:, 512 * g:512 * (g + 1)].T).astype(BF)
        in_maps.append({
            "xT": xT[b], "wqkvT": wqkvT, "woT": woT,
            "cosT": cosT, "sinT": sinT, "rotm": rotm, "ident": ident,
            "ones1": ones1, "onesr": onesr,
        })

    trace = bool(int(os.environ.get("KERNEL_TRACE", "0")))
    out = run_bass_kernel_spmd(nc, in_maps, list(range(N_CORES)), trace=trace)
    LAST_EXEC_NS = out.exec_time_ns
    LAST_RESULTS = out
    y = np.zeros((B, S, HID), dtype=np.float32)
    for c in range(N_CORES):
        b = c // TPG
        y[b] += np.asarray(out.results[c]["y"]).astype(np.float32)
    return y


# revision 5
# speedup vs baseline: 1.2991x; 1.2991x over previous
"""Gated-attention (Qwen-style) Trainium2 kernel — fp16, scheduling-optimized.

Sharding (8 cores): data-parallel over batch (2) x tensor-parallel over head
groups (4). Core c handles batch b=c//4 and head group g=c%4: q heads
4g..4g+3, kv heads 2g..2g+1, gate logits 4g..4g+3, w_o columns 512g..512g+512.
Each core computes a partial output y_g = attn_out_g @ w_o[:, cols_g].T in
fp16; the host sums the 4 partials per batch in f32.

v3 design notes (baseline was ACT-bound in phase 2 with PE HAM oscillation):
- Phase 1 qkv projection in stationary groups of 2 output tiles; rope /
  v-transpose processing of group g is emitted after the matmuls of group
  g+1, so PE never stalls on the PSUM->SBUF casts (which run on ACT).
  The gate PSUM shares the rope-rotation pool (tag-level rotation, bufs=2).
- Phase 2 j-loop is software-pipelined: scores+exp for j-pair jp are emitted
  one step ahead of the PV matmuls of jp-1, with out-projection matmuls of
  block i-1 interleaved 2-per-step as PE filler (exp latency cover).
- exp runs on [128,1024] PSUM score-pair tiles (one exp per j-pair).
- e-sums accumulate on [128,1024] pairs; one [128,512] fold add feeds the
  ones-matmul denominator.
- psos (PV PSUM) drains to SBUF fp16 immediately after each j-loop, freeing
  PSUM banks and decoupling the deferred denominator chain; the gated scale
  multiply is then an fp16 TT op.
- den/bc matmuls are deferred and interleaved with direct out-projection
  tiles so their DVE/ACT dependency chains resolve off the PE critical path.
"""

import os
from contextlib import ExitStack

import numpy as np

B, S, HID = 2, 2048, 2048
NH, NKV, HD = 16, 8, 128
GATE = NH
KV_DIM = NKV * HD

N_CORES = 8
TPG = 4            # tensor-parallel group size (head groups)
QH = NH // TPG     # q heads per core = 4
KVH = NKV // TPG   # kv heads per core = 2
IB = 512           # phase-1 token block
NB = S // IB       # 4 blocks
JT = S // 128      # 16 key tiles
JP = JT // 2       # 8 key tile-pairs
IBLK = 512         # phase-2 query block
NI = S // IBLK     # 4 query blocks
WCOL = 1032        # packed qkv+gate weight cols (1028 used, padded)
SCALE = 1.0 / float(np.sqrt(HD))

_CACHE = {}

LAST_EXEC_NS = None
LAST_RESULTS = None


def _build_program():
    import concourse.bass as bass
    import concourse.mybir as mybir
    from concourse import bacc
    from concourse.tile import TileContext

    F32 = mybir.dt.float32
    F32R = mybir.dt.float32r
    F16 = mybir.dt.float16
    AF = mybir.ActivationFunctionType

    nc = bacc.Bacc()

    xT_d = nc.dram_tensor("xT", [HID, S], F16, kind="ExternalInput")
    wqkvT_d = nc.dram_tensor("wqkvT", [HID, WCOL], F16, kind="ExternalInput")
    woT_d = nc.dram_tensor("woT", [QH * HD, HID], F16, kind="ExternalInput")
    cosT_d = nc.dram_tensor("cosT", [HD, S], F32, kind="ExternalInput")
    sinT_d = nc.dram_tensor("sinT", [HD, S], F32, kind="ExternalInput")
    rotm_d = nc.dram_tensor("rotm", [HD, HD], F16, kind="ExternalInput")
    ident_d = nc.dram_tensor("ident", [128, 128], F16, kind="ExternalInput")
    ones1_d = nc.dram_tensor("ones1", [128, 1], F16, kind="ExternalInput")
    onesr_d = nc.dram_tensor("onesr", [1, 128], F32R, kind="ExternalInput")
    y_d = nc.dram_tensor("y", [S, HID], F16, kind="ExternalOutput")

    with TileContext(nc) as tc, ExitStack() as persist:
        const = persist.enter_context(tc.tile_pool(name="const", bufs=1))
        rotm_sb = const.tile([HD, HD], F16, tag="rotm", name="rotm")
        nc.scalar.dma_start(out=rotm_sb, in_=rotm_d[:, :])
        ident_sb = const.tile([128, 128], F16, tag="ident", name="ident")
        nc.scalar.dma_start(out=ident_sb, in_=ident_d[:, :])
        ones1_sb = const.tile([128, 1], F16, tag="ones1", name="ones1")
        nc.scalar.dma_start(out=ones1_sb, in_=ones1_d[:, :])
        onesr_sb = const.tile([1, 128], F32R, tag="onesr", name="onesr")
        nc.scalar.dma_start(out=onesr_sb, in_=onesr_d[:, :])

        # weights on ACT/DVE sequencers so x loads own the SP/Pool DGEs
        wpool = persist.enter_context(tc.tile_pool(name="w", bufs=1))
        wsb = [wpool.tile([128, WCOL], F16, tag=f"w{h}", name=f"w{h}") for h in range(16)]
        for h in range(16):
            nc.scalar.dma_start(out=wsb[h], in_=wqkvT_d[128 * h:128 * (h + 1), :])
        cos_sb = const.tile([HD, S], F32, tag="cos", name="cos")
        nc.scalar.dma_start(out=cos_sb, in_=cosT_d[:, :])
        sin_sb = const.tile([HD, S], F32, tag="sin", name="sin")
        nc.scalar.dma_start(out=sin_sb, in_=sinT_d[:, :])
        wopool = persist.enter_context(tc.tile_pool(name="wo", bufs=1))
        wo_sb = [wopool.tile([128, HID], F16, tag=f"wo{i}", name=f"wo{i}") for i in range(4)]

        qk_pool = persist.enter_context(tc.tile_pool(name="qk", bufs=1))
        qk_sb = [qk_pool.tile([128, S], F16, tag=f"qk{r}", name=f"qk{r}") for r in range(QH + KVH)]
        v_pool = persist.enter_context(tc.tile_pool(name="v", bufs=1))
        v_sb = [v_pool.tile([128, KVH * HD], F16, tag=f"v{t}", name=f"v{t}") for t in range(JT)]
        g_pool = persist.enter_context(tc.tile_pool(name="g", bufs=1))
        sgflat = g_pool.tile([1, QH * S], F32, tag="sgflat", name="sgflat")

        # ---------------- phase 1: qkv projection + rope + v transpose -----
        with ExitStack() as ph1:
            xpool = ph1.enter_context(tc.tile_pool(name="x", bufs=32))
            tmppool = ph1.enter_context(tc.tile_pool(name="tmp", bufs=3))
            vrawpool = ph1.enter_context(tc.tile_pool(name="vraw", bufs=2))
            sgpool = ph1.enter_context(tc.tile_pool(name="sg", bufs=1))

            ps_acc = ph1.enter_context(tc.tile_pool(name="acc", bufs=4, space="PSUM"))
            ps_rot = ph1.enter_context(tc.tile_pool(name="rot", bufs=2, space="PSUM"))
            ps_tp = ph1.enter_context(tc.tile_pool(name="tp", bufs=2, space="PSUM"))

            pending_proc = [None]

            def emit_pending():
                if pending_proc[0] is not None:
                    pending_proc[0]()
                    pending_proc[0] = None

            for ib in range(NB):
                sl = slice(IB * ib, IB * (ib + 1))
                xb = []
                for h in range(16):
                    xt = xpool.tile([128, IB], F16, tag="x", name="x")
                    eng = nc.gpsimd if h % 2 == 0 else nc.sync
                    eng.dma_start(out=xt, in_=xT_d[128 * h:128 * (h + 1), sl])
                    xb.append(xt)

                # gate logits PSUM shares the rope-rotation pool (tag "rot")
                psg_full = ps_rot.tile([128, IB], F32, tag="rot", name="psg")
                psg = psg_full[0:QH, :]
                for h in range(16):
                    nc.tensor.matmul(psg, wsb[h][:, 1024:1024 + QH], xb[h],
                                     start=(h == 0), stop=(h == 15))
                # previous block's last group processing fills the PE here
                emit_pending()
                # sigmoid per block, flattened to partition 0 for phase 2
                eT = sgpool.tile([QH, IB], F32, tag="eT", name="eT")
                nc.scalar.activation(out=eT, in_=psg, func=AF.Exp, scale=-1.0)
                nc.vector.tensor_scalar_add(eT, eT, 1.0)
                sgT = sgpool.tile([QH, IB], F32, tag="sgT", name="sgT")
                nc.vector.reciprocal_approx_fast(out=sgT, in_=eT)
                for h in range(QH):
                    nc.sync.dma_start(out=sgflat[0:1, S * h + IB * ib:S * h + IB * (ib + 1)],
                                      in_=sgT[h:h + 1, :])

                # 4 stationary groups of 2 output row-tiles each:
                # (q0,q1) (q2,q3) (k0,k1) (v0,v1)
                for grp in range(4):
                    accs = [ps_acc.tile([128, IB], F32, tag="acc", name="acc")
                            for _ in range(2)]
                    for h in range(16):
                        for r2 in range(2):
                            r = 2 * grp + r2
                            nc.tensor.matmul(
                                accs[r2], wsb[h][:, 128 * r:128 * (r + 1)], xb[h],
                                start=(h == 0), stop=(h == 15))

                    def make_proc(grp, accs, sl):
                        def proc():
                            for r2 in range(2):
                                r = 2 * grp + r2
                                if r < QH + KVH:  # q or k row-tile: rope
                                    craw = tmppool.tile([128, IB], F16, tag="craw", name="craw")
                                    nc.scalar.copy(craw, accs[r2])
                                    rps = ps_rot.tile([128, IB], F32, tag="rot", name="rot")
                                    nc.tensor.matmul(rps, rotm_sb, craw, start=True, stop=True)
                                    t1 = tmppool.tile([128, IB], F32R, tag="t1", name="t1")
                                    nc.vector.tensor_mul(t1, accs[r2], cos_sb[:, sl])
                                    t2 = tmppool.tile([128, IB], F32R, tag="t2", name="t2")
                                    nc.vector.tensor_mul(t2, rps, sin_sb[:, sl])
                                    nc.vector.tensor_add(qk_sb[r][:, sl], t1, t2)
                                else:  # v row-tile: transpose to [tokens, d]
                                    vraw = vrawpool.tile([128, IB], F16, tag="vraw", name="vraw")
                                    nc.scalar.copy(vraw, accs[r2])
                                    vh = r - (QH + KVH)
                                    ibb = (sl.start // IB)
                                    for s2 in range(IB // 128):
                                        tp = ps_tp.tile([128, 128], F16, tag="tp", name="tp")
                                        nc.tensor.transpose(
                                            tp, vraw[:, 128 * s2:128 * (s2 + 1)], ident_sb)
                                        tt = (IB // 128) * ibb + s2
                                        nc.vector.tensor_copy(
                                            v_sb[tt][:, 128 * vh:128 * (vh + 1)], tp)
                        return proc

                    emit_pending()
                    pending_proc[0] = make_proc(grp, accs, sl)

            for cc in range(4):
                nc.gpsimd.dma_start(out=wo_sb[cc], in_=woT_d[128 * cc:128 * (cc + 1), :])
            emit_pending()

        # ---------------- phase 2: attention + gate + out-projection -------
        with ExitStack() as ph2:
            oc_pool = ph2.enter_context(tc.tile_pool(name="oc", bufs=1))
            OC = [oc_pool.tile([128, S], F16, tag=f"oc{h}", name=f"oc{h}") for h in range(QH)]
            epool = ph2.enter_context(tc.tile_pool(name="e", bufs=4))
            accpool = ph2.enter_context(tc.tile_pool(name="dacc", bufs=4))
            popool = ph2.enter_context(tc.tile_pool(name="po", bufs=4))
            scpool = ph2.enter_context(tc.tile_pool(name="sc", bufs=2))
            foldpool = ph2.enter_context(tc.tile_pool(name="fold", bufs=4))
            ypool = ph2.enter_context(tc.tile_pool(name="y", bufs=3))

            ps_s = ph2.enter_context(tc.tile_pool(name="pss", bufs=2, space="PSUM"))
            ps_o = ph2.enter_context(tc.tile_pool(name="pso", bufs=2, space="PSUM"))
            ps_sh = ph2.enter_context(tc.tile_pool(name="pssh", bufs=2, space="PSUM"))

            def oproj_steps(t, drain_all_dve):
                """out-projection for token tile t as 8 closures of ~2 MMs each."""
                state = {}

                def start():
                    state["ysb"] = ypool.tile([128, HID], F16, tag="y", name="y")

                steps = []
                for o in range(4):
                    def s_a(o=o):
                        if o == 0:
                            start()
                        state[o] = ps_sh.tile([128, IBLK], F32, tag="sh", name="psy")
                        for cc in range(2):
                            nc.tensor.matmul(
                                state[o], OC[cc][:, 128 * t:128 * (t + 1)],
                                wo_sb[cc][:, IBLK * o:IBLK * (o + 1)],
                                start=(cc == 0), stop=False)

                    def s_b(o=o):
                        for cc in range(2, 4):
                            nc.tensor.matmul(
                                state[o], OC[cc][:, 128 * t:128 * (t + 1)],
                                wo_sb[cc][:, IBLK * o:IBLK * (o + 1)],
                                start=False, stop=(cc == 3))
                        ysb = state["ysb"]
                        dst = ysb[:, IBLK * o:IBLK * (o + 1)]
                        if drain_all_dve or o != 0:
                            nc.vector.tensor_copy(dst, state[o])
                        else:
                            nc.scalar.copy(dst, state[o])
                        if o == 3:
                            nc.gpsimd.dma_start(
                                out=y_d[128 * t:128 * (t + 1), :], in_=ysb)

                    steps.append(s_a)
                    steps.append(s_b)
                return steps

            def emit_jloop(i, kv, fillers):
                """software-pipelined scores->exp->PV loop; fillers: list of
                closures (PE micro-steps) consumed one per pipeline slot."""
                isl = slice(IBLK * i, IBLK * (i + 1))
                psos = []
                accs2 = []
                for hh in range(2):
                    psos.append(ps_o.tile([128, IBLK], F32, tag="pso", name="pso"))
                    accs2.append(accpool.tile([128, 2 * IBLK], F16, tag="dacc", name="dacc"))
                fill_iter = iter(fillers)
                es_prev = None
                for jp in range(JP + 1):
                    es = None
                    if jp < JP:
                        es = []
                        for hh in range(2):
                            h = 2 * kv + hh
                            pss = ps_s.tile([128, 2 * IBLK], F32, tag="pss", name="pss")
                            for jj in range(2):
                                j = 2 * jp + jj
                                jsl = slice(128 * j, 128 * (j + 1))
                                nc.tensor.matmul(pss[:, IBLK * jj:IBLK * (jj + 1)],
                                                 qk_sb[QH + kv][:, jsl],
                                                 qk_sb[h][:, isl], start=True, stop=True)
                            e = epool.tile([128, 2 * IBLK], F16, tag="e", name="e")
                            nc.scalar.activation(out=e, in_=pss, func=AF.Exp, scale=SCALE)
                            es.append(e)
                    if jp >= 1:
                        jq = jp - 1
                        for hh in range(2):
                            if jq == 0:
                                nc.vector.tensor_copy(accs2[hh], es_prev[hh])
                            else:
                                nc.vector.tensor_add(accs2[hh], accs2[hh], es_prev[hh])
                            for jj in range(2):
                                j = 2 * jq + jj
                                nc.tensor.matmul(psos[hh],
                                                 v_sb[j][:, 128 * kv:128 * (kv + 1)],
                                                 es_prev[hh][:, IBLK * jj:IBLK * (jj + 1)],
                                                 start=(j == 0), stop=(j == JT - 1))
                        step = next(fill_iter, None)
                        if step is not None:
                            step()
                    es_prev = es
                # drain PV PSUM to SBUF fp16 (frees banks; decouples den chain)
                psout = []
                for hh in range(2):
                    po = popool.tile([128, IBLK], F16, tag="po", name="po")
                    nc.vector.tensor_copy(po, psos[hh])
                    psout.append(po)
                # fold the e-sum pairs early on DVE (feeds deferred den matmul)
                folds = []
                for hh in range(2):
                    fold = foldpool.tile([128, IBLK], F16, tag="fold", name="fold")
                    nc.vector.tensor_add(fold, accs2[hh][:, :IBLK], accs2[hh][:, IBLK:])
                    folds.append(fold)
                # leftover fillers (i==0 loops have none queued anyway)
                for step in fill_iter:
                    step()
                return psout, folds

            def emit_den_mms(kv, folds, dens):
                for hh in range(2):
                    den = ps_sh.tile([1, IBLK], F32, tag="sh", name="den")
                    nc.tensor.matmul(den, ones1_sb, folds[hh], start=True, stop=True)
                    dens.append(den)

            def emit_sc(i, kv, dens):
                """DVE part of the den chain (recip + gate mul); emitted
                BEFORE any ps_sh reuse so WAR waits resolve off-PE."""
                scs = []
                for hh in range(2):
                    h = 2 * kv + hh
                    rec = scpool.tile([1, IBLK], F32, tag="rec", name="rec")
                    nc.vector.reciprocal_approx_fast(out=rec, in_=dens[hh])
                    sc = scpool.tile([1, IBLK], F32R, tag="sc", name="sc")
                    nc.vector.tensor_mul(
                        sc, rec, sgflat[0:1, S * h + IBLK * i:S * h + IBLK * (i + 1)])
                    scs.append(sc)
                return scs

            def emit_bc_oc(i, kv, psout, scs):
                isl = slice(IBLK * i, IBLK * (i + 1))
                for hh in range(2):
                    h = 2 * kv + hh
                    bc = ps_sh.tile([128, IBLK], F32, tag="sh", name="bc")
                    nc.tensor.matmul(bc, onesr_sb, scs[hh], start=True, stop=True)
                    bcs = scpool.tile([128, IBLK], F16, tag="bcs", name="bcs")
                    nc.scalar.copy(bcs, bc)
                    nc.vector.tensor_mul(OC[h][:, isl], psout[hh], bcs)

            def emit_oproj_direct(t, cover=None):
                steps = oproj_steps(t, drain_all_dve=False)
                out = []
                for idx, step in enumerate(steps):
                    step()
                    if cover is not None and idx == 3:
                        cover()
                return out

            # emission schedule
            for i in range(NI):
                f0 = oproj_steps(4 * (i - 1) + 2, True) if i > 0 else []
                po0, fo0 = emit_jloop(i, 0, f0)
                f1 = oproj_steps(4 * (i - 1) + 3, True) if i > 0 else []
                po1, fo1 = emit_jloop(i, 1, f1)
                dens0 = []
                emit_den_mms(0, fo0, dens0)
                scs0 = emit_sc(i, 0, dens0)
                if i > 0:
                    # direct tile covers the recip/sc DVE latency of kv0
                    emit_oproj_direct(4 * (i - 1) + 0)
                emit_bc_oc(i, 0, po0, scs0)
                dens1 = []
                emit_den_mms(1, fo1, dens1)
                scs1 = emit_sc(i, 1, dens1)
                if i > 0:
                    emit_oproj_direct(4 * (i - 1) + 1)
                emit_bc_oc(i, 1, po1, scs1)
            for t in (12, 13, 14, 15):
                emit_oproj_direct(t)

    nc.finalize()
    return nc


def kernel(hidden_states, cos, sin, w_qkv, w_o):
    global LAST_EXEC_NS, LAST_RESULTS
    from concourse.bass_utils import run_bass_kernel_spmd

    BF = np.float16
    hidden_states = np.asarray(hidden_states, dtype=np.float32)
    cos = np.asarray(cos, dtype=np.float32)
    sin = np.asarray(sin, dtype=np.float32)
    w_qkv = np.asarray(w_qkv, dtype=np.float32)
    w_o = np.asarray(w_o, dtype=np.float32)

    if "nc" not in _CACHE:
        _CACHE["nc"] = _build_program()
    nc = _CACHE["nc"]

    cosT = np.ascontiguousarray(cos.T)
    sinT = np.ascontiguousarray(sin.T)
    rotm = np.zeros((HD, HD), dtype=np.float32)
    for i in range(HD // 2):
        rotm[i + HD // 2, i] = -1.0   # rot[d'] = -q[d'+64] for d' < 64
        rotm[i, i + HD // 2] = 1.0    # rot[d'] = +q[d'-64] for d' >= 64
    rotm = rotm.astype(BF)
    ident = np.eye(128, dtype=np.float32).astype(BF)
    ones1 = np.ones((128, 1), dtype=np.float16)
    onesr = np.ones((1, 128), dtype=np.float32)

    xT = [np.ascontiguousarray(hidden_states[b].T).astype(BF) for b in range(B)]
    in_maps = []
    for c in range(N_CORES):
        b, g = divmod(c, TPG)
        qr = w_qkv[512 * g:512 * (g + 1)]
        kr = w_qkv[HID + GATE + 256 * g:HID + GATE + 256 * (g + 1)]
        vr = w_qkv[HID + GATE + KV_DIM + 256 * g:HID + GATE + KV_DIM + 256 * (g + 1)]
        gr = w_qkv[HID + QH * g:HID + QH * (g + 1)]
        pad = np.zeros((WCOL - 1024 - QH, HID), dtype=np.float32)
        wqkvT = np.ascontiguousarray(
            np.concatenate([qr, kr, vr, gr, pad], axis=0).T).astype(BF)
        woT = np.ascontiguousarray(w_o[:, 512 * g:512 * (g + 1)].T).astype(BF)
        in_maps.append({
            "xT": xT[b], "wqkvT": wqkvT, "woT": woT,
            "cosT": cosT, "sinT": sinT, "rotm": rotm, "ident": ident,
            "ones1": ones1, "onesr": onesr,
        })

    trace = bool(int(os.environ.get("KERNEL_TRACE", "0")))
    out = run_bass_kernel_spmd(nc, in_maps, list(range(N_CORES)), trace=trace)
    LAST_EXEC_NS = out.exec_time_ns
    LAST_RESULTS = out
    y = np.zeros((B, S, HID), dtype=np.float32)
    for c in range(N_CORES):
        b = c // TPG
        y[b] += np.asarray(out.results[c]["y"]).astype(np.float32)
    return y
